# revision 1
# baseline (speedup 1.0000x reference)
"""Single-head causal attention (B=16, S=2048, D=1024, H=128) on 8 TRN2 cores.

Batch-parallel across cores (2 batches per core), weights replicated, bf16
compute with fp32 PSUM accumulation.

v2 design (post-trace):
  - X is uploaded host-side as bf16 ALREADY in the transposed SBUF layout
    XT[p, c, s] = X[s, c*128 + p]; staging is pure DMA (half the bytes of
    fp32, no DVE cast, no on-device transpose).
  - Per 512-wide q window: projection (Q/K/V = W^T X^T on PE, c-outer
    accumulation) immediately followed by flash-style attention in
    transposed layout with causal block skipping. Window epilogues are
    deferred past the NEXT window's projection so PE never stalls on the
    softmax-denominator/normalization chain.
  - Softmax denominator: e-tiles pair-summed on GPSIMD, merged with a
    binary-counter tree on DVE, ONE [128->1, 512] ones-matmul per window
    (the baseline's per-pair den matmuls were 13us of PE).
  - Engine budget: PE matmuls/transposes; ACT(scalar) does exp ONLY (it
    sets the attention cadence at ~550ns/tile); DVE evicts psums, merges
    the den tree, normalizes; GPSIMD pair-sums + diagonal zero-fills.
  - Dummy warmup matmuls hold the PE HAM clock up through the DMA lead-in.
"""

import numpy as np
import ml_dtypes

import concourse.bass as bass
import concourse.bacc as bacc
import concourse.mybir as mybir
from concourse import tile
from concourse.bass_utils import run_bass_kernel_spmd

F32 = mybir.dt.float32
BF16 = mybir.dt.bfloat16
PSUM = bass.MemorySpace.PSUM
Exp = mybir.ActivationFunctionType.Exp

P = 128          # partition dim / head size / tile unit
D = 1024         # model dim
H = 128          # head size
DW = D // P      # 8 d-groups
N_CORES = 8
N_WARMUP = 10    # kernel-start PE warmup matmuls


def build_nc(BSH, S, SW=512):
    """Build the per-core Bass program. BSH = batches per core."""
    NW = S // SW      # q windows
    NT = S // P       # 128-row tiles in S
    WPT = SW // P     # q tiles per window
    ISQ = float(1.0 / np.sqrt(H))

    nc = bacc.Bacc("TRN2", target_bir_lowering=False, debug=False)

    # Layouts are chosen so every DMA moves long per-partition-contiguous
    # runs: the DMA engines are descriptor-bound (~1KB rows cost ~90ns
    # each regardless of size), so window-major X (8KB runs), a combined
    # weight tensor (6KB runs) and window-major output (2KB runs) beat the
    # naive layouts by ~4x on effective bandwidth.
    x_d = nc.dram_tensor("x", [BSH, P, NW, DW, SW], BF16,
                         kind="ExternalInput")
    w_d = nc.dram_tensor("wqkv", [P, 3, DW, H], BF16, kind="ExternalInput")
    mask_d = nc.dram_tensor("mask", [P, P], BF16, kind="ExternalInput")
    id32_d = nc.dram_tensor("id32", [P, P], F32, kind="ExternalInput")
    id16_d = nc.dram_tensor("id16", [P, P], BF16, kind="ExternalInput")
    ones_d = nc.dram_tensor("ones", [P, 1], BF16, kind="ExternalInput")
    out_d = nc.dram_tensor("out", [BSH, NW, P, WPT, H], F32,
                           kind="ExternalOutput")

    with tile.TileContext(nc) as tc:
        from contextlib import ExitStack

        with ExitStack() as ctx:
            cpool = ctx.enter_context(tc.tile_pool(name="consts", bufs=1))
            big = ctx.enter_context(tc.tile_pool(name="big", bufs=2))

            # warmup feedstock first: no DMA dependency
            z16 = cpool.tile([P, SW], BF16, tag="z16")
            nc.gpsimd.memset(z16[:], 0.0)

            XT, QT, KT, VT, V = {}, {}, {}, {}, {}
            for b in range(BSH):
                XT[b] = big.tile([P, NW, DW, SW], BF16, tag="xt",
                                 name=f"xt{b}")
                QT[b] = big.tile([P, S], BF16, tag="qt", name=f"qt{b}")
                KT[b] = big.tile([P, S], BF16, tag="kt", name=f"kt{b}")
                # VT fp32: its only consumer is the PE transpose, and fp32
                # transposes land in fp32 PSUM, which DVE can evict
                # (bf16 PSUM is scalar-engine-only)
                VT[b] = big.tile([P, S], F32, tag="vt", name=f"vt{b}")
                V[b] = big.tile([P, NT, P], BF16, tag="v", name=f"v{b}")

            # weights: single DMA, 6KB/partition contiguous
            w_sb = cpool.tile([P, 3, DW, H], BF16, tag="wqkv")
            nc.scalar.dma_start(w_sb[:], w_d.ap())
            mask_sb = cpool.tile([P, P], BF16, tag="mask")
            nc.scalar.dma_start(mask_sb[:], mask_d.ap())
            id32_sb = cpool.tile([P, P], F32, tag="id32")
            nc.scalar.dma_start(id32_sb[:], id32_d.ap())
            id16_sb = cpool.tile([P, P], BF16, tag="id16")
            nc.scalar.dma_start(id16_sb[:], id16_d.ap())
            ones_sb = cpool.tile([P, 1], BF16, tag="ones")
            nc.scalar.dma_start(ones_sb[:], ones_d.ap())

            # X: window-major, 8KB/partition contiguous per window chunk;
            # b0 per-window for pipeline start, later batches whole (32KB
            # runs)
            for w in range(NW):
                nc.sync.dma_start(XT[0][:, w, :, :], x_d.ap()[0, :, w, :, :])
            for b in range(1, BSH):
                nc.sync.dma_start(XT[b][:], x_d.ap()[b])

            # ---- PE warmup: HAM starts at K=4/8 (1.2 GHz); dummy matmuls
            # during the DMA lead-in get the clock to 8/8 before real work.
            with tc.tile_pool(name="warm", bufs=1, space=PSUM) as wp:
                wps = wp.tile([P, SW], F32, tag="w")
                for _ in range(N_WARMUP):
                    nc.tensor.matmul(wps[:], z16[:, 0:P], z16[:],
                                     start=True, stop=True)

            epool = ctx.enter_context(tc.tile_pool(name="exp", bufs=6))
            tpool = ctx.enter_context(tc.tile_pool(name="tree", bufs=8))
            opool = ctx.enter_context(tc.tile_pool(name="osb", bufs=3))
            spool = ctx.enter_context(tc.tile_pool(name="small", bufs=2))

            # persistent PSUM pools; slots are bank-granular (2KB/part per
            # tile): pj 2 + scp 3 + oup 1 + trp 2 = 8 banks. The
            # den-transpose borrows scp tiles (it runs between windows when
            # the scores ring is idle).
            pj = ctx.enter_context(tc.tile_pool(name="pj", bufs=2, space=PSUM))
            scp = ctx.enter_context(tc.tile_pool(name="scp", bufs=3, space=PSUM))
            oup = ctx.enter_context(tc.tile_pool(name="oup", bufs=1, space=PSUM))
            trp = ctx.enter_context(tc.tile_pool(name="trp", bufs=2, space=PSUM))

            # ---- per-window projection, split so attention-tail work can
            # slot between the Q and K/V parts ----
            def proj_1(b, w, widx, dst):
                lo, hi = w * SW, (w + 1) * SW
                ps = pj.tile([P, SW], F32, tag="ps")
                for c in range(DW):
                    nc.tensor.matmul(
                        ps[:], w_sb[:, widx, c, :], XT[b][:, w, c, :],
                        start=(c == 0), stop=(c == DW - 1),
                    )
                nc.vector.tensor_copy(dst[:, lo:hi], ps[:])

            def proj_q(b, w):
                proj_1(b, w, 0, QT[b])

            def proj_kv(b, w):
                proj_1(b, w, 1, KT[b])
                proj_1(b, w, 2, VT[b])
                for t in range(w * WPT, (w + 1) * WPT):
                    vp = trp.tile([P, P], F32, tag="tr")
                    nc.tensor.transpose(
                        vp[:], VT[b][:, t * P:(t + 1) * P], id32_sb[:]
                    )
                    nc.vector.tensor_copy(V[b][:, t, :], vp[:])

            # ---- attention window body (scores/exp/out + den tree) ----
            ST = {}   # (b, w) -> (outp, den_tile)

            def attn_body(b, w):
                nj = WPT * (w + 1)
                outp = oup.tile([P, SW], F32, tag="o")
                sp = {}

                def scores(j):
                    c0 = max(0, j - WPT * w)
                    s = scp.tile([P, SW], F32, tag="s")
                    nc.tensor.matmul(
                        s[:, c0 * P:],
                        KT[b][:, j * P:(j + 1) * P],
                        QT[b][:, w * SW + c0 * P:(w + 1) * SW],
                        start=True, stop=True,
                    )
                    sp[j] = s

                pair_in = []   # e tiles awaiting a pair-sum
                tree = []      # partial sums, tree depth capped at quads:
                #                the den matmul accumulates over all roots,
                #                so deep (slow, serial) merge chains are
                #                replaced by a few extra 1-column matmuls

                def step(j, first, last, defer):
                    c0 = max(0, j - WPT * w)
                    e = epool.tile([P, SW], BF16, tag="e")
                    if c0 > 0:
                        nc.gpsimd.memset(e[:, 0:c0 * P], 0.0)
                    nc.scalar.activation(
                        e[:, c0 * P:], sp[j][:, c0 * P:], Exp, scale=ISQ
                    )
                    if j >= WPT * w:
                        nc.vector.tensor_mul(
                            e[:, c0 * P:(c0 + 1) * P],
                            e[:, c0 * P:(c0 + 1) * P],
                            mask_sb[:],
                        )

                    def emit():
                        nc.tensor.matmul(
                            outp[:, (0 if first else c0 * P):],
                            V[b][:, j, :], e[:, (0 if first else c0 * P):],
                            start=first, stop=last,
                        )

                    # pairs on DVE (they gate e-ring reuse and must match
                    # the exp cadence); quad merges on otherwise-idle gpsimd
                    pair_in.append(e)
                    if len(pair_in) == 2:
                        pr = tpool.tile([P, SW], BF16, tag="pr")
                        nc.vector.tensor_add(
                            pr[:], pair_in[0][:], pair_in[1][:])
                        pair_in.clear()
                        if tree and tree[-1][0] == 1:
                            _, prev = tree.pop()
                            nt = tpool.tile([P, SW], BF16, tag="pr")
                            # last window's final quad gates the run tail:
                            # DVE (~.44us) there instead of gpsimd (~1.15us)
                            qadd = nc.vector.tensor_add if (
                                b == BSH - 1 and w == NW - 1
                            ) else nc.gpsimd.tensor_add
                            qadd(nt[:], prev[:], pr[:])
                            tree.append((2, nt))
                        else:
                            tree.append((1, pr))

                    if defer:
                        return emit
                    emit()
                    return None

                # diagonal tiles FIRST: their memset/mask/short-exp chain
                # lands while the engine queues are empty, and the window's
                # trailing (deferred) tiles become mask-free full tiles.
                order = list(range(WPT * w, nj)) + list(range(WPT * w))
                # the first emitted tile is diagonal j=WPT*w (c0=0), so the
                # accumulation group still opens full-width
                ndefer = 2 if w > 0 else 0
                scores(order[0])
                rets = []
                for i, j in enumerate(order):
                    if i + 1 < nj:
                        scores(order[i + 1])
                    rets.append(
                        step(j, first=(i == 0), last=(i == nj - 1),
                             defer=(i >= nj - ndefer)))
                ST[(b, w)] = (outp, [t for _, t in tree])

                def tail():
                    for emit in rets[nj - ndefer:]:
                        emit()

                return tail

            # ---- window epilogue: den matmuls, transpose back, normalize
            def attn_epi(b, w):
                outp, roots = ST.pop((b, w))
                # den TRANSPOSED directly: dtr[q,1] = root[:,qblock]^T @ 1,
                # PSUM-accumulated over the tree roots. 1-column matmuls are
                # ~55ns, so extra roots beat deep serial merge chains.
                dtt = scp.tile([P, SW], F32, tag="s")
                dtr = dtt[:, 0:WPT]
                nr = len(roots)
                for c in range(WPT):
                    for ri, root in enumerate(roots):
                        nc.tensor.matmul(
                            dtr[:, c:c + 1], root[:, c * P:(c + 1) * P],
                            ones_sb[:], start=(ri == 0), stop=(ri == nr - 1),
                        )
                rec = spool.tile([P, WPT], F32, tag="rec")
                nc.vector.reciprocal(rec[:], dtr[:])
                oT = spool.tile([P, SW], F32, tag="oT")
                nc.vector.tensor_copy(oT[:], outp[:])
                osb = opool.tile([P, WPT, P], F32, tag="osb")
                for c in range(WPT):
                    otr = trp.tile([P, P], F32, tag="tr")
                    nc.tensor.transpose(
                        otr[:], oT[:, c * P:(c + 1) * P], id32_sb[:]
                    )
                    nc.vector.tensor_scalar_mul(
                        osb[:, c, :], otr[:], rec[:, c:c + 1]
                    )
                # osb [P, WPT, H] matches out_d's [b, w] slice exactly:
                # 2KB/partition contiguous descriptors
                nc.scalar.dma_start(out_d.ap()[b, w], osb[:])

            # ---- driver: the window joint is interleaved so PE never
            # waits on the exp/merge chains:
            #   body(w) | projQ(w+1) | tail(w) | projKV(w+1) | epi(w) |
            #   body(w+1) ...
            tail_fn, pend_epi = None, None
            for b in range(BSH):
                for w in range(NW):
                    proj_q(b, w)
                    if tail_fn is not None:
                        tail_fn()
                    proj_kv(b, w)
                    if pend_epi is not None:
                        attn_epi(*pend_epi)
                    tail_fn = attn_body(b, w)
                    pend_epi = (b, w)
            tail_fn()
            attn_epi(*pend_epi)

    nc.compile()
    return nc


def make_consts():
    bf16 = ml_dtypes.bfloat16
    mask = np.triu(np.ones((P, P), dtype=np.float32)).astype(bf16)
    id32 = np.eye(P, dtype=np.float32)
    id16 = np.eye(P, dtype=np.float32).astype(bf16)
    ones = np.ones((P, 1), dtype=np.float32).astype(bf16)
    return mask, id32, id16, ones


def prep_weights(Wq, Wk, Wv):
    """-> [P, 3, DW, H] bf16: weight row d=c*P+p sits at [p, i, c, h]."""
    bf16 = ml_dtypes.bfloat16
    w = np.stack([np.asarray(W, dtype=np.float32).reshape(DW, P, H)
                  for W in (Wq, Wk, Wv)])          # [3, DW, P, H]
    return np.ascontiguousarray(w.transpose(2, 0, 1, 3)).astype(bf16)


def prep_x(x16, SW=512):
    """[BSH, S, D] bf16 -> [BSH, P, NW, DW, SW]: x[b,p,w,c,sw] =
    X[b, w*SW+sw, c*P+p]. Window-major so each window stages as one DMA of
    8KB-contiguous per-partition runs."""
    BSH, S, D_ = x16.shape
    NW = S // SW
    return np.ascontiguousarray(
        x16.reshape(BSH, NW, SW, DW, P).transpose(0, 4, 1, 3, 2))


_NC_CACHE = {}


def _get_nc(BSH, S, SW=512):
    key = (BSH, S, SW)
    if key not in _NC_CACHE:
        _NC_CACHE[key] = build_nc(BSH, S, SW)
    return _NC_CACHE[key]


def make_in_maps(input, Wq, Wk, Wv):
    input = np.asarray(input, dtype=np.float32)
    B, S, D_ = input.shape
    assert D_ == D and B % N_CORES == 0
    BSH = B // N_CORES
    wqkv = prep_weights(Wq, Wk, Wv)
    mask, id32, id16, ones = make_consts()
    x16 = input.astype(ml_dtypes.bfloat16)
    in_maps = []
    for i in range(N_CORES):
        m = {
            "x": prep_x(x16[i * BSH:(i + 1) * BSH]),
            "wqkv": wqkv,
            "mask": mask, "id32": id32, "id16": id16, "ones": ones,
        }
        in_maps.append(m)
    return in_maps, BSH, S


def kernel(input, Wq, Wk, Wv):
    in_maps, BSH, S = make_in_maps(input, Wq, Wk, Wv)
    nc = _get_nc(BSH, S)
    res = run_bass_kernel_spmd(nc, in_maps, core_ids=list(range(N_CORES)))
    # out_d is [BSH, NW, P, WPT, H]: q = w*SW + c*P + r -> [b, w, c, r, h]
    outs = []
    for i in range(N_CORES):
        o = res.results[i]["out"]          # [BSH, NW, P, WPT, H]
        BSHl, NW, Pl, WPT, Hl = o.shape
        outs.append(o.transpose(0, 1, 3, 2, 4).reshape(BSHl, NW * WPT * Pl, Hl))
    return np.concatenate(outs, axis=0)

